# revision 21
# baseline (speedup 1.0000x reference)
"""Trainium2 Bass kernel for the MACE-style symmetric contraction:

    out  = einsum("xyik,kc,bci->bcxy", U3, w3, nf)
    c2   = einsum("xyk,kc->cxy", U2, w2)[None] + out
    out  = einsum("bcxi,bci->bcx", c2, nf)
    c1   = einsum("xk,kc->cx", U1, w1)[None] + out
    out  = einsum("bci,bci->bc", c1, nf)

Algebraically:

    out[b,c] =   sum_{x,y,i} W3U[x,y,i,c] nf[b,c,x] nf[b,c,y] nf[b,c,i]
               + sum_{x,y}   U2w2[c,x,y]  nf[b,c,x] nf[b,c,y]
               + sum_{x}     U1w1[c,x]    nf[b,c,x]

with W3U = einsum("xyik,kc->xyic", U3, w3).  U2/U1 fold into the triple
product via an augmented i row (i'=48 holds U2w2; (i'=48,y'=48) holds
U1w1) and a constant-1 channel.

Sharding: irrep axis x (48) split 6-per-core across 8 NeuronCores (this
splits the dominant HBM stream, U3, 8 ways).  Host sums the 8 partial
[512, 96] outputs.

Device pipeline (all fp16 except PSUM/scan state, which are fp32):
  build:   W3U[c, (i,x,y')] = w3.T @ u3t on PE, k-accumulated in PSUM,
           drained fp16 to a DRAM scratch.
  phase B: per c-pair + b-tile: Z[b,(x,y')] = nfa.T @ W3U_c on PE; then
           a fused multiply+prefix-sum (custom DVE MAC-scan, or
           ACT-drain + Pool multiply + Pool scan) gives running sums of
           Z*nfy whose row boundaries are the per-x group sums.  The
           per-x difference of boundaries is folded into the host-side
           dnfx = nf[x] - nf[x+1] (Abel summation), so
           out[b,c] = sum_x bnd[b,c,x] * dnfx[b,c,x].
"""

import numpy as np

B = 512          # atoms
C = 96           # feats
I = 48           # irreps
K3, K2, K1 = 1270, 24, 3
NCORES = 8
XS = I // NCORES  # 6 x-values per core
Y1 = I + 1        # 49: y plus augmentation column
I1 = I + 1        # 49: i plus augmentation row
KP = 1280         # K3 padded to 10 partition tiles
NX = XS * Y1      # 294
MP = I * XS * Y1  # 14112  (m = (i, x, y'), i outermost)
MCHUNK = 1024
NMC = (MP + MCHUNK - 1) // MCHUNK  # 14 (last chunk 800)
KT = KP // 128                     # 10
PAIRS = C // 2                     # 48
NT = B // 128                      # 4 b-tiles

_CACHE = {}

# Per-c-pair phase-B path: 'D' = DVE MAC-scan, 'Q' = ACT+Pool+DVE-reduce.
# Must match between _build_nc (engine choice) and _prep_inputs (final
# weights: dnfx for 'D', nfx for 'Q').
_PATTERN = ['D' if (cp % 8) < 3 else 'Q' for cp in range(PAIRS)]

# exec time of the last device run (ns), when BASS_TRACE=1
LAST_EXEC_NS = None


def _register_mac_scan():
    """Custom DVE op: out[t] = prefix-sum of in0[t]*in1[t] (fp32 state).
    Fuses phase B's elementwise multiply and its y'-group reduction into
    one DVE pass; group sums are recovered from the running sum at row
    boundaries."""
    import concourse.dve_ops as dve_ops_mod
    if any(op.name == "TT_MAC_SCAN_ANT" for op in dve_ops_mod.OPS):
        return next(op for op in dve_ops_mod.OPS
                    if op.name == "TT_MAC_SCAN_ANT")
    from concourse.dve_spec import Spec, scan, Src0, Src1
    from concourse.dve_uop import AluOp
    from concourse.dve_ops import DveOp

    def _ref_mac_scan(in0, in1, s0, s1, imm2):
        p = in0.astype(np.float32) * in1.astype(np.float32)
        return np.cumsum(p.reshape(p.shape[0], -1), axis=1).reshape(
            p.shape).astype(np.float32)

    spec = Spec(body=scan(AluOp.ADD, Src0 * Src1), reference=_ref_mac_scan)
    op = DveOp("TT_MAC_SCAN_ANT", spec, subdim=False,
               uops_sha={"v3": "b3fc3e78a862b7eb",
                         "v4": "bc6a002865d48b97"})
    dve_ops_mod.OPS.append(op)
    dve_ops_mod.CUSTOM_DVE_SPECS[op.name] = spec
    dve_ops_mod._SUB_OPCODE_FOR_NAME[op.name] = (
        max(dve_ops_mod._SUB_OPCODE_FOR_NAME.values()) + 1)
    return op


def _build_nc(debug=None):
    import concourse.bass as bass
    import concourse.mybir as mybir
    from concourse.tile import TileContext

    mac_scan = _register_mac_scan()

    f16 = mybir.dt.float16
    f32 = mybir.dt.float32
    mult = mybir.AluOpType.mult
    add = mybir.AluOpType.add
    bypass = mybir.AluOpType.bypass

    import concourse.bacc as bacc
    nc = bacc.Bacc(None, target_bir_lowering=False)
    u3t = nc.dram_tensor("u3t", [KP, MP], f16, kind="ExternalInput")
    w3p = nc.dram_tensor("w3p", [KP, C], f16, kind="ExternalInput")
    nfa = nc.dram_tensor("nfa", [128, PAIRS * B], f16, kind="ExternalInput")
    nfy = nc.dram_tensor("nfy", [B, C * I1], f16, kind="ExternalInput")
    # final-pass weights: dnfx (Abel) for 'D' c-pairs, plain nfx for 'Q'
    # c-pairs -- host-built to match PATTERN
    fwx = nc.dram_tensor("fwx", [B, C * XS], f16, kind="ExternalInput")
    u2aug = nc.dram_tensor("u2aug", [32, NX], f16, kind="ExternalInput")
    w21 = nc.dram_tensor("w21", [32, C], f16, kind="ExternalInput")
    outp = nc.dram_tensor("out", [B, C], f32, kind="ExternalOutput")

    with TileContext(nc) as tc:
        with (
            nc.allow_low_precision(reason="fp16 intermediates; rel-err "
                                   "budget 2e-2 vs ~1e-3 incurred"),
            tc.tile_pool(name="dram", bufs=1, space="DRAM") as dpool,
            tc.tile_pool(name="const", bufs=1) as cpool,
            tc.tile_pool(name="u3", bufs=10) as u3pool,
            tc.tile_pool(name="bpsum", bufs=2, space="PSUM") as bpsum,
            tc.tile_pool(name="zpsum", bufs=2, space="PSUM") as zpsum,
            tc.tile_pool(name="lt", bufs=3) as ltpool,
            tc.tile_pool(name="sc", bufs=4) as scpool,
            tc.tile_pool(name="tm", bufs=4) as tmpool,
            tc.tile_pool(name="zs", bufs=4) as zspool,
            tc.tile_pool(name="stg", bufs=3) as stgpool,
            tc.tile_pool(name="fin", bufs=2) as finpool,
        ):
            # scratch row c = [(i'=0..47) from the U3 build | (i'=48) aug]
            w3u_scr = dpool.tile([C, I1 * NX], f16)

            # ---- resident constants ----
            w3sb = cpool.tile([128, KT * C], f16)
            w3v = w3sb[:, :].rearrange("p (k c) -> p k c", c=C)
            nc.sync.dma_start(
                out=w3v[:, :, :],
                in_=w3p[:, :].rearrange("(k p) c -> p k c", p=128))
            nfasb = cpool.tile([128, PAIRS * B], f16)
            nc.sync.dma_start(out=nfasb[:, :], in_=nfa[:, :])
            nfav = nfasb[:, :].rearrange("p (cp b) -> p cp b", b=B)
            w21sb = cpool.tile([32, C], f16)
            nc.sync.dma_start(out=w21sb[:, :], in_=w21[:, :])
            u2sb = cpool.tile([32, NX], f16)
            nc.sync.dma_start(out=u2sb[:, :], in_=u2aug[:, :])
            nfyts = [cpool.tile([128, C * I1], f16, tag=f"nfy{t}",
                                name=f"nfy{t}") for t in range(NT)]
            fwxts = [cpool.tile([128, C * XS], f16, tag=f"fwx{t}",
                                name=f"fwx{t}") for t in range(NT)]
            ybufs = [cpool.tile([128, C * XS], f32, tag=f"yb{t}",
                                name=f"yb{t}") for t in range(NT)]
            for t in range(NT):
                nc.sync.dma_start(out=nfyts[t][:, :],
                                  in_=nfy[t * 128:(t + 1) * 128, :])
                nc.sync.dma_start(out=fwxts[t][:, :],
                                  in_=fwx[t * 128:(t + 1) * 128, :])

            # ---- aug build: [96, 294] = w21.T @ u2aug ----
            aps = bpsum.tile([C, 512], f32, tag="bp")
            nc.tensor.matmul(aps[:, :NX], w21sb[:27, :], u2sb[:27, :],
                             start=True, stop=True)
            astg = stgpool.tile([C, MCHUNK], f16, tag="stg")
            nc.scalar.copy(astg[:, :NX], aps[:, :NX])
            nc.sync.dma_start(out=w3u_scr[:, I * NX:I1 * NX],
                              in_=astg[:, :NX])

            # ---- W3U build: [96, 14112] = w3p.T @ u3t, k-accumulated ----
            for mc in range(NMC):
                w = min(MCHUNK, MP - mc * MCHUNK)
                h1 = min(512, w)
                h2 = w - h1
                ps = bpsum.tile([C, MCHUNK], f32, tag="bp", name=f"bp{mc}")
                for kt in range(KT):
                    t = u3pool.tile([128, MCHUNK], f16, tag="u3")
                    # two dma_starts per tile -> two queues in parallel
                    nc.sync.dma_start(
                        out=t[:, :h1],
                        in_=u3t[kt * 128:(kt + 1) * 128,
                                mc * MCHUNK:mc * MCHUNK + h1])
                    if h2 > 0:
                        nc.sync.dma_start(
                            out=t[:, 512:w],
                            in_=u3t[kt * 128:(kt + 1) * 128,
                                    mc * MCHUNK + 512:mc * MCHUNK + w])
                    nc.tensor.matmul(ps[:, :h1], w3v[:, kt, :], t[:, :h1],
                                     start=(kt == 0), stop=(kt == KT - 1))
                    if h2 > 0:
                        nc.tensor.matmul(ps[:, 512:w], w3v[:, kt, :],
                                         t[:, 512:w],
                                         start=(kt == 0), stop=(kt == KT - 1))
                stg = stgpool.tile([C, MCHUNK], f16, tag="stg")
                nc.scalar.copy(stg[:, :w], ps[:, :w])
                nc.sync.dma_start(
                    out=w3u_scr[:, mc * MCHUNK:mc * MCHUNK + w],
                    in_=stg[:, :w])

            # ---- phase B ----
            w3u_v = w3u_scr[:, :].rearrange("c (i xy) -> c i xy", xy=NX)
            if debug == "A":
                npairs = 0
            elif isinstance(debug, int):
                npairs = debug
            else:
                npairs = PAIRS
            # Per-c-pair engine assignment (uniform across t so each
            # ybufs column slice has one consistent semantic):
            #   'D' = DVE MAC-scans from PSUM -> boundary cums (dnfx final)
            #   'Q' = ACT drain + Pool multiply + DVE reduce -> true group
            #         sums (plain nfx final)
            pattern = _PATTERN
            for cp in range(npairs):
                c0, c1 = 2 * cp, 2 * cp + 1
                lt = ltpool.tile([128, NX], f16, tag="lt")
                nc.sync.dma_start(out=lt[0:I1, :], in_=w3u_v[c0])
                nc.sync.dma_start(out=lt[64:64 + I1, :], in_=w3u_v[c1])
                for t in range(NT):
                    nfyv = nfyts[t][:, c0 * I1:(c1 + 1) * I1].rearrange(
                        "p (c i) -> p c i", i=I1)
                    zt = zpsum.tile([128, 1024], f32, tag="z")
                    for ci in range(2):
                        lhsT = nfav[64 * ci:64 * ci + I1, cp,
                                    t * 128:(t + 1) * 128]
                        nc.tensor.matmul(zt[:, 512 * ci:512 * ci + NX], lhsT,
                                         lt[64 * ci:64 * ci + I1, :],
                                         start=True, stop=True)
                    ybv = ybufs[t][:, cp * 2 * XS:(cp + 1) * 2 * XS]
                    if pattern[cp] == 'D':
                        # fused multiply+scan per c; boundary cums -> ybufs
                        sc = scpool.tile([128, 2 * NX], f32, tag="sc")
                        for ci in range(2):
                            zv3 = zt[:, 512 * ci:512 * ci + NX].rearrange(
                                "p (x y) -> p x y", y=Y1)
                            nfb = nfyv[:, ci, None, :].to_broadcast(
                                [128, XS, Y1])
                            ov = sc[:, ci * NX:(ci + 1) * NX].rearrange(
                                "p (x y) -> p x y", y=Y1)
                            nc.vector._custom_dve(
                                mac_scan, out=ov, in0=zv3, in1=nfb)
                        bnd = sc[:, :].rearrange(
                            "p (c x y) -> p c x y", c=2, y=Y1)[:, :, :, I]
                        nc.scalar.copy(
                            ybv.rearrange("p (c x) -> p c x", c=2), bnd)
                    else:  # 'Q'
                        zs = zspool.tile([128, 2 * NX], f16, tag="zs")
                        zsv = zs[:, :].rearrange("p (c x y) -> p c x y",
                                                 c=2, y=Y1)
                        zv = zt[:, :].rearrange(
                            "p (c n) -> p c n", n=512)[:, :, 0:NX].rearrange(
                            "p c (x y) -> p c x y", y=Y1)
                        nc.scalar.copy(zsv, zv)
                        tmp = tmpool.tile([128, 2 * NX], f16, tag="tm")
                        tv = tmp[:, :].rearrange("p (c x y) -> p c x y",
                                                 c=2, y=Y1)
                        nfyb = nfyv[:, :, None, :].to_broadcast(
                            [128, 2, XS, Y1])
                        nc.gpsimd.tensor_tensor(tv, zsv, nfyb, mult)
                        nc.vector.tensor_reduce(
                            ybv.rearrange("p (c x) -> p c x", c=2), tv,
                            axis=mybir.AxisListType.X, op=add)
            if debug != "A":
                for t in range(NT):
                    dnv = fwxts[t][:, :].rearrange("p (c x) -> p c x", x=XS)
                    ybv = ybufs[t][:, :].rearrange("p (c x) -> p c x", x=XS)
                    yn = finpool.tile([128, C * XS], f32, tag="yn")
                    ynv = yn[:, :].rearrange("p (c x) -> p c x", x=XS)
                    nc.vector.tensor_tensor(ynv, ybv, dnv, mult)
                    ostf = finpool.tile([128, C], f32, tag="ostf")
                    nc.vector.tensor_reduce(
                        ostf[:, :], ynv, axis=mybir.AxisListType.X, op=add)
                    nc.sync.dma_start(out=outp[t * 128:(t + 1) * 128, :],
                                      in_=ostf[:, :])
    nc.finalize()
    return nc


def _prep_inputs(node_feats, w3, w2, w1, U3, U2, U1):
    """Host-side sharding / re-layout: transposes, dtype casts, padding,
    concatenation, and the Abel-summation difference of adjacent nf_x."""
    f16 = np.float16
    f32 = np.float32
    node_feats = np.asarray(node_feats, dtype=f32)
    nf16 = node_feats.astype(f16)

    # shared across cores
    w3p = np.zeros((KP, C), dtype=f16)
    w3p[:K3] = np.asarray(w3, dtype=f32).astype(f16)
    w21 = np.zeros((32, C), dtype=f16)
    w21[:K2] = np.asarray(w2, dtype=f32).astype(f16)
    w21[K2:K2 + K1] = np.asarray(w1, dtype=f32).astype(f16)

    # nfa: [p, cp, b]; p = 64*(c%2) + i'; i'=48 row is the ones channel
    nfT = nf16.transpose(1, 2, 0)  # [c, i, b]
    nfa = np.zeros((128, PAIRS, B), dtype=f16)
    for par in (0, 1):
        nfa[64 * par:64 * par + I] = nfT[par::2].transpose(1, 0, 2)
        nfa[64 * par + I] = 1.0
    nfa = np.ascontiguousarray(nfa.reshape(128, PAIRS * B))

    # nfy: [b, c, 49] = nf with ones channel
    nfy = np.empty((B, C, I1), dtype=f16)
    nfy[:, :, :I] = nf16
    nfy[:, :, I] = 1.0
    nfy = np.ascontiguousarray(nfy.reshape(B, C * I1))

    # One shared fp16 transpose of U3 to [k, i, x, y], then per-core
    # x-slice + y-pad + k-pad.
    U3_16 = np.asarray(U3, dtype=f32).astype(f16)
    u3_kixy = np.ascontiguousarray(U3_16.transpose(3, 2, 0, 1))  # [k,i,x,y]
    U2_16 = np.asarray(U2, dtype=f32).astype(f16)
    U1_16 = np.asarray(U1, dtype=f32).astype(f16)

    in_maps = []
    for r in range(NCORES):
        xlo = XS * r
        u3a = np.zeros((KP, I, XS, Y1), dtype=f16)
        u3a[:K3, :, :, :I] = u3_kixy[:, :, xlo:xlo + XS, :]
        u3t = np.ascontiguousarray(u3a.reshape(KP, MP))

        # u2aug: rows 0:24 U2 slice, rows 24:27 U1 slice (at y'=48)
        u2a = np.zeros((32, XS, Y1), dtype=f16)
        u2a[:K2, :, :I] = U2_16[xlo:xlo + XS].transpose(2, 0, 1)
        u2a[K2:K2 + K1, :, I] = U1_16[xlo:xlo + XS].T
        u2a = np.ascontiguousarray(u2a.reshape(32, NX))

        # Final-pass weights per c-pair path.  'D' pairs get Abel weights
        # (device wrote boundary cums): out_c = sum_x bnd[x]*dnfx[x] with
        # dnfx[b,c,x] = nf[b,c,xlo+x] - nf[b,c,xlo+x+1], nf[...,xlo+XS]=0.
        # 'Q' pairs get plain nfx (device wrote true group sums).
        sl = node_feats[:, :, xlo:xlo + XS]
        fwx = np.empty((B, C, XS), dtype=f32)
        for cp in range(PAIRS):
            cols = slice(2 * cp, 2 * cp + 2)
            if _PATTERN[cp] == 'D':
                fwx[:, cols, :XS - 1] = (sl[:, cols, :XS - 1]
                                         - sl[:, cols, 1:])
                fwx[:, cols, XS - 1] = sl[:, cols, XS - 1]
            else:
                fwx[:, cols, :] = sl[:, cols, :]
        fwx = np.ascontiguousarray(fwx.astype(f16).reshape(B, C * XS))

        in_maps.append({
            "u3t": u3t,
            "w3p": w3p,
            "nfa": nfa,
            "nfy": nfy,
            "fwx": fwx,
            "u2aug": u2a,
            "w21": w21,
        })
    return in_maps


def kernel(node_feats, w3, w2, w1, U3, U2, U1):
    global LAST_EXEC_NS
    import os
    from concourse.bass_utils import run_bass_kernel_spmd

    if "nc" not in _CACHE:
        _CACHE["nc"] = _build_nc()
    nc = _CACHE["nc"]

    in_maps = _prep_inputs(node_feats, w3, w2, w1, U3, U2, U1)
    trace = bool(os.environ.get("BASS_TRACE"))
    res = run_bass_kernel_spmd(nc, in_maps, list(range(NCORES)), trace=trace)
    LAST_EXEC_NS = res.exec_time_ns
    _CACHE["last_results"] = res

    out = np.zeros((B, C), dtype=np.float64)
    for r in range(NCORES):
        out += res.results[r]["out"].astype(np.float64)
    return out.astype(np.float32)


# revision 24
# speedup vs baseline: 1.1619x; 1.1619x over previous
"""Trainium2 Bass kernel for the MACE-style symmetric contraction:

    out  = einsum("xyik,kc,bci->bcxy", U3, w3, nf)
    c2   = einsum("xyk,kc->cxy", U2, w2)[None] + out
    out  = einsum("bcxi,bci->bcx", c2, nf)
    c1   = einsum("xk,kc->cx", U1, w1)[None] + out
    out  = einsum("bci,bci->bc", c1, nf)

Algebraically:

    out[b,c] =   sum_{x,y,i} W3U[x,y,i,c] nf[b,c,x] nf[b,c,y] nf[b,c,i]
               + sum_{x,y}   U2w2[c,x,y]  nf[b,c,x] nf[b,c,y]
               + sum_{x}     U1w1[c,x]    nf[b,c,x]

with W3U = einsum("xyik,kc->xyic", U3, w3).  U2/U1 fold into the triple
product via an augmented i row (i'=48 holds U2w2; (i'=48,y'=48) holds
U1w1) and a constant-1 channel.

Sharding: irrep axis x (48) split 6-per-core across 8 NeuronCores (this
splits the dominant HBM stream, U3, 8 ways).  Host sums the 8 partial
[512, 96] outputs.

Device pipeline (all fp16 except PSUM/scan state, which are fp32):
  build:   W3U[c, (i,x,y')] = w3.T @ u3t on PE, k-accumulated in PSUM,
           drained fp16 to a DRAM scratch.
  phase B: per c-pair + b-tile: Z[b,(x,y')] = nfa.T @ W3U_c on PE; then
           a fused multiply+prefix-sum (custom DVE MAC-scan, or
           ACT-drain + Pool multiply + Pool scan) gives running sums of
           Z*nfy whose row boundaries are the per-x group sums.  The
           per-x difference of boundaries is folded into the host-side
           dnfx = nf[x] - nf[x+1] (Abel summation), so
           out[b,c] = sum_x bnd[b,c,x] * dnfx[b,c,x].
"""

import numpy as np

B = 512          # atoms
C = 96           # feats
I = 48           # irreps
K3, K2, K1 = 1270, 24, 3
NCORES = 8
XS = I // NCORES  # 6 x-values per core
Y1 = I + 1        # 49: y plus augmentation column
I1 = I + 1        # 49: i plus augmentation row
KP = 1280         # K3 padded to 10 partition tiles
NX = XS * Y1      # 294
MP = I * XS * Y1  # 14112  (m = (i, x, y'), i outermost)
MCHUNK = 1024
NMC = (MP + MCHUNK - 1) // MCHUNK  # 14 (last chunk 800)
KT = KP // 128                     # 10
PAIRS = C // 2                     # 48
NT = B // 128                      # 4 b-tiles

_CACHE = {}

# Per-c-pair phase-B path: 'D' = DVE MAC-scan, 'Q' = ACT+Pool+DVE-reduce.
# Must match between _build_nc (engine choice) and _prep_inputs (final
# weights: dnfx for 'D', nfx for 'Q').
_PATTERN = ['D' if (cp % 8) in (0, 3, 6) else 'Q' for cp in range(PAIRS)]

# exec time of the last device run (ns), when BASS_TRACE=1
LAST_EXEC_NS = None


def _register_mac_scan():
    """Custom DVE op: out[t] = prefix-sum of in0[t]*in1[t] (fp32 state).
    Fuses phase B's elementwise multiply and its y'-group reduction into
    one DVE pass; group sums are recovered from the running sum at row
    boundaries."""
    import concourse.dve_ops as dve_ops_mod
    if any(op.name == "TT_MAC_SCAN_ANT" for op in dve_ops_mod.OPS):
        return next(op for op in dve_ops_mod.OPS
                    if op.name == "TT_MAC_SCAN_ANT")
    from concourse.dve_spec import Spec, scan, Src0, Src1
    from concourse.dve_uop import AluOp
    from concourse.dve_ops import DveOp

    def _ref_mac_scan(in0, in1, s0, s1, imm2):
        p = in0.astype(np.float32) * in1.astype(np.float32)
        return np.cumsum(p.reshape(p.shape[0], -1), axis=1).reshape(
            p.shape).astype(np.float32)

    spec = Spec(body=scan(AluOp.ADD, Src0 * Src1), reference=_ref_mac_scan)
    op = DveOp("TT_MAC_SCAN_ANT", spec, subdim=False,
               uops_sha={"v3": "b3fc3e78a862b7eb",
                         "v4": "bc6a002865d48b97"})
    dve_ops_mod.OPS.append(op)
    dve_ops_mod.CUSTOM_DVE_SPECS[op.name] = spec
    dve_ops_mod._SUB_OPCODE_FOR_NAME[op.name] = (
        max(dve_ops_mod._SUB_OPCODE_FOR_NAME.values()) + 1)
    return op


def _build_nc(debug=None):
    import concourse.bass as bass
    import concourse.mybir as mybir
    from concourse.tile import TileContext

    mac_scan = _register_mac_scan()

    f16 = mybir.dt.float16
    f32 = mybir.dt.float32
    mult = mybir.AluOpType.mult
    add = mybir.AluOpType.add
    bypass = mybir.AluOpType.bypass

    import concourse.bacc as bacc
    nc = bacc.Bacc(None, target_bir_lowering=False)
    u3t = nc.dram_tensor("u3t", [KP, MP], f16, kind="ExternalInput")
    w3p = nc.dram_tensor("w3p", [KP, C], f16, kind="ExternalInput")
    nfa = nc.dram_tensor("nfa", [128, PAIRS * B], f16, kind="ExternalInput")
    nfy = nc.dram_tensor("nfy", [B, C * I1], f16, kind="ExternalInput")
    # final-pass weights: dnfx (Abel) for 'D' c-pairs, plain nfx for 'Q'
    # c-pairs -- host-built to match PATTERN
    fwx = nc.dram_tensor("fwx", [B, C * XS], f16, kind="ExternalInput")
    u2aug = nc.dram_tensor("u2aug", [32, NX], f16, kind="ExternalInput")
    w21 = nc.dram_tensor("w21", [32, C], f16, kind="ExternalInput")
    outp = nc.dram_tensor("out", [B, C], f32, kind="ExternalOutput")

    with TileContext(nc) as tc:
        with (
            nc.allow_low_precision(reason="fp16 intermediates; rel-err "
                                   "budget 2e-2 vs ~1e-3 incurred"),
            tc.tile_pool(name="dram", bufs=1, space="DRAM") as dpool,
            tc.tile_pool(name="const", bufs=1) as cpool,
            tc.tile_pool(name="u3", bufs=12) as u3pool,
            tc.tile_pool(name="bpsum", bufs=1, space="PSUM") as bpsum,
            tc.tile_pool(name="zpsum", bufs=3, space="PSUM") as zpsum,
            tc.tile_pool(name="lt", bufs=4) as ltpool,
            tc.tile_pool(name="sc", bufs=6) as scpool,
            tc.tile_pool(name="tm", bufs=6) as tmpool,
            tc.tile_pool(name="zs", bufs=6) as zspool,
            tc.tile_pool(name="stg", bufs=3) as stgpool,
            tc.tile_pool(name="fin", bufs=2) as finpool,
        ):
            # scratch row c = [(i'=0..47) from the U3 build | (i'=48) aug]
            w3u_scr = dpool.tile([C, I1 * NX], f16)

            # ---- resident constants ----
            w3sb = cpool.tile([128, KT * C], f16)
            w3v = w3sb[:, :].rearrange("p (k c) -> p k c", c=C)
            nc.sync.dma_start(
                out=w3v[:, :, :],
                in_=w3p[:, :].rearrange("(k p) c -> p k c", p=128))
            nfasb = cpool.tile([128, PAIRS * B], f16)
            nc.sync.dma_start(out=nfasb[:, :], in_=nfa[:, :])
            nfav = nfasb[:, :].rearrange("p (cp b) -> p cp b", b=B)
            w21sb = cpool.tile([32, C], f16)
            nc.sync.dma_start(out=w21sb[:, :], in_=w21[:, :])
            u2sb = cpool.tile([32, NX], f16)
            nc.sync.dma_start(out=u2sb[:, :], in_=u2aug[:, :])
            nfyts = [cpool.tile([128, C * I1], f16, tag=f"nfy{t}",
                                name=f"nfy{t}") for t in range(NT)]
            fwxts = [cpool.tile([128, C * XS], f16, tag=f"fwx{t}",
                                name=f"fwx{t}") for t in range(NT)]
            ybufs = [cpool.tile([128, C * XS], f32, tag=f"yb{t}",
                                name=f"yb{t}") for t in range(NT)]
            for t in range(NT):
                nc.sync.dma_start(out=nfyts[t][:, :],
                                  in_=nfy[t * 128:(t + 1) * 128, :])
                nc.sync.dma_start(out=fwxts[t][:, :],
                                  in_=fwx[t * 128:(t + 1) * 128, :])

            # ---- aug build: [96, 294] = w21.T @ u2aug ----
            aps = bpsum.tile([C, 512], f32, tag="bp")
            nc.tensor.matmul(aps[:, :NX], w21sb[:27, :], u2sb[:27, :],
                             start=True, stop=True)
            astg = stgpool.tile([C, MCHUNK], f16, tag="stg")
            nc.scalar.copy(astg[:, :NX], aps[:, :NX])
            nc.sync.dma_start(out=w3u_scr[:, I * NX:I1 * NX],
                              in_=astg[:, :NX])

            # ---- W3U build: [96, 14112] = w3p.T @ u3t, k-accumulated ----
            for mc in range(NMC):
                w = min(MCHUNK, MP - mc * MCHUNK)
                h1 = min(512, w)
                h2 = w - h1
                ps = bpsum.tile([C, MCHUNK], f32, tag="bp", name=f"bp{mc}")
                for kt in range(KT):
                    t = u3pool.tile([128, MCHUNK], f16, tag="u3")
                    nc.sync.dma_start(
                        out=t[:, :w],
                        in_=u3t[kt * 128:(kt + 1) * 128,
                                mc * MCHUNK:mc * MCHUNK + w])
                    nc.tensor.matmul(ps[:, :h1], w3v[:, kt, :], t[:, :h1],
                                     start=(kt == 0), stop=(kt == KT - 1))
                    if h2 > 0:
                        nc.tensor.matmul(ps[:, 512:w], w3v[:, kt, :],
                                         t[:, 512:w],
                                         start=(kt == 0), stop=(kt == KT - 1))
                stg = stgpool.tile([C, MCHUNK], f16, tag="stg")
                nc.scalar.copy(stg[:, :w], ps[:, :w])
                nc.sync.dma_start(
                    out=w3u_scr[:, mc * MCHUNK:mc * MCHUNK + w],
                    in_=stg[:, :w])

            # ---- phase B ----
            w3u_v = w3u_scr[:, :].rearrange("c (i xy) -> c i xy", xy=NX)
            if debug == "A":
                npairs = 0
            elif isinstance(debug, int):
                npairs = debug
            else:
                npairs = PAIRS
            # Per-c-pair engine assignment (uniform across t so each
            # ybufs column slice has one consistent semantic):
            #   'D' = DVE MAC-scans from PSUM -> boundary cums (dnfx final)
            #   'Q' = ACT drain + Pool multiply + DVE reduce -> true group
            #         sums (plain nfx final)
            pattern = _PATTERN
            for cp in range(npairs):
                c0, c1 = 2 * cp, 2 * cp + 1
                lt = ltpool.tile([128, NX], f16, tag="lt")
                nc.sync.dma_start(out=lt[0:I1, :], in_=w3u_v[c0])
                nc.sync.dma_start(out=lt[64:64 + I1, :], in_=w3u_v[c1])
                for t in range(NT):
                    nfyv = nfyts[t][:, c0 * I1:(c1 + 1) * I1].rearrange(
                        "p (c i) -> p c i", i=I1)
                    zt = zpsum.tile([128, 1024], f32, tag="z")
                    for ci in range(2):
                        lhsT = nfav[64 * ci:64 * ci + I1, cp,
                                    t * 128:(t + 1) * 128]
                        nc.tensor.matmul(zt[:, 512 * ci:512 * ci + NX], lhsT,
                                         lt[64 * ci:64 * ci + I1, :],
                                         start=True, stop=True)
                    ybv = ybufs[t][:, cp * 2 * XS:(cp + 1) * 2 * XS]
                    if pattern[cp] == 'D':
                        # fused multiply+scan per c; boundary cums -> ybufs
                        sc = scpool.tile([128, 2 * NX], f32, tag="sc")
                        for ci in range(2):
                            zv3 = zt[:, 512 * ci:512 * ci + NX].rearrange(
                                "p (x y) -> p x y", y=Y1)
                            nfb = nfyv[:, ci, None, :].to_broadcast(
                                [128, XS, Y1])
                            ov = sc[:, ci * NX:(ci + 1) * NX].rearrange(
                                "p (x y) -> p x y", y=Y1)
                            nc.vector._custom_dve(
                                mac_scan, out=ov, in0=zv3, in1=nfb)
                        bnd = sc[:, :].rearrange(
                            "p (c x y) -> p c x y", c=2, y=Y1)[:, :, :, I]
                        nc.scalar.copy(
                            ybv.rearrange("p (c x) -> p c x", c=2), bnd)
                    else:  # 'Q'
                        zs = zspool.tile([128, 2 * NX], f16, tag="zs")
                        zsv = zs[:, :].rearrange("p (c x y) -> p c x y",
                                                 c=2, y=Y1)
                        zv = zt[:, :].rearrange(
                            "p (c n) -> p c n", n=512)[:, :, 0:NX].rearrange(
                            "p c (x y) -> p c x y", y=Y1)
                        nc.scalar.copy(zsv, zv)
                        tmp = tmpool.tile([128, 2 * NX], f16, tag="tm")
                        tv = tmp[:, :].rearrange("p (c x y) -> p c x y",
                                                 c=2, y=Y1)
                        nfyb = nfyv[:, :, None, :].to_broadcast(
                            [128, 2, XS, Y1])
                        nc.gpsimd.tensor_tensor(tv, zsv, nfyb, mult)
                        nc.vector.tensor_reduce(
                            ybv.rearrange("p (c x) -> p c x", c=2), tv,
                            axis=mybir.AxisListType.X, op=add)
            if debug != "A":
                for t in range(NT):
                    dnv = fwxts[t][:, :].rearrange("p (c x) -> p c x", x=XS)
                    ybv = ybufs[t][:, :].rearrange("p (c x) -> p c x", x=XS)
                    yn = finpool.tile([128, C * XS], f32, tag="yn")
                    ynv = yn[:, :].rearrange("p (c x) -> p c x", x=XS)
                    nc.vector.tensor_tensor(ynv, ybv, dnv, mult)
                    ostf = finpool.tile([128, C], f32, tag="ostf")
                    nc.vector.tensor_reduce(
                        ostf[:, :], ynv, axis=mybir.AxisListType.X, op=add)
                    nc.sync.dma_start(out=outp[t * 128:(t + 1) * 128, :],
                                      in_=ostf[:, :])
    nc.finalize()
    return nc


def _prep_inputs(node_feats, w3, w2, w1, U3, U2, U1):
    """Host-side sharding / re-layout: transposes, dtype casts, padding,
    concatenation, and the Abel-summation difference of adjacent nf_x."""
    f16 = np.float16
    f32 = np.float32
    node_feats = np.asarray(node_feats, dtype=f32)
    nf16 = node_feats.astype(f16)

    # shared across cores
    w3p = np.zeros((KP, C), dtype=f16)
    w3p[:K3] = np.asarray(w3, dtype=f32).astype(f16)
    w21 = np.zeros((32, C), dtype=f16)
    w21[:K2] = np.asarray(w2, dtype=f32).astype(f16)
    w21[K2:K2 + K1] = np.asarray(w1, dtype=f32).astype(f16)

    # nfa: [p, cp, b]; p = 64*(c%2) + i'; i'=48 row is the ones channel
    nfT = nf16.transpose(1, 2, 0)  # [c, i, b]
    nfa = np.zeros((128, PAIRS, B), dtype=f16)
    for par in (0, 1):
        nfa[64 * par:64 * par + I] = nfT[par::2].transpose(1, 0, 2)
        nfa[64 * par + I] = 1.0
    nfa = np.ascontiguousarray(nfa.reshape(128, PAIRS * B))

    # nfy: [b, c, 49] = nf with ones channel
    nfy = np.empty((B, C, I1), dtype=f16)
    nfy[:, :, :I] = nf16
    nfy[:, :, I] = 1.0
    nfy = np.ascontiguousarray(nfy.reshape(B, C * I1))

    # One shared fp16 transpose of U3 to [k, i, x, y], then per-core
    # x-slice + y-pad + k-pad.
    U3_16 = np.asarray(U3, dtype=f32).astype(f16)
    u3_kixy = np.ascontiguousarray(U3_16.transpose(3, 2, 0, 1))  # [k,i,x,y]
    U2_16 = np.asarray(U2, dtype=f32).astype(f16)
    U1_16 = np.asarray(U1, dtype=f32).astype(f16)

    in_maps = []
    for r in range(NCORES):
        xlo = XS * r
        u3a = np.zeros((KP, I, XS, Y1), dtype=f16)
        u3a[:K3, :, :, :I] = u3_kixy[:, :, xlo:xlo + XS, :]
        u3t = np.ascontiguousarray(u3a.reshape(KP, MP))

        # u2aug: rows 0:24 U2 slice, rows 24:27 U1 slice (at y'=48)
        u2a = np.zeros((32, XS, Y1), dtype=f16)
        u2a[:K2, :, :I] = U2_16[xlo:xlo + XS].transpose(2, 0, 1)
        u2a[K2:K2 + K1, :, I] = U1_16[xlo:xlo + XS].T
        u2a = np.ascontiguousarray(u2a.reshape(32, NX))

        # Final-pass weights per c-pair path.  'D' pairs get Abel weights
        # (device wrote boundary cums): out_c = sum_x bnd[x]*dnfx[x] with
        # dnfx[b,c,x] = nf[b,c,xlo+x] - nf[b,c,xlo+x+1], nf[...,xlo+XS]=0.
        # 'Q' pairs get plain nfx (device wrote true group sums).
        sl = node_feats[:, :, xlo:xlo + XS]
        fwx = np.empty((B, C, XS), dtype=f32)
        for cp in range(PAIRS):
            cols = slice(2 * cp, 2 * cp + 2)
            if _PATTERN[cp] == 'D':
                fwx[:, cols, :XS - 1] = (sl[:, cols, :XS - 1]
                                         - sl[:, cols, 1:])
                fwx[:, cols, XS - 1] = sl[:, cols, XS - 1]
            else:
                fwx[:, cols, :] = sl[:, cols, :]
        fwx = np.ascontiguousarray(fwx.astype(f16).reshape(B, C * XS))

        in_maps.append({
            "u3t": u3t,
            "w3p": w3p,
            "nfa": nfa,
            "nfy": nfy,
            "fwx": fwx,
            "u2aug": u2a,
            "w21": w21,
        })
    return in_maps


def kernel(node_feats, w3, w2, w1, U3, U2, U1):
    global LAST_EXEC_NS
    import os
    from concourse.bass_utils import run_bass_kernel_spmd

    if "nc" not in _CACHE:
        _CACHE["nc"] = _build_nc()
    nc = _CACHE["nc"]

    in_maps = _prep_inputs(node_feats, w3, w2, w1, U3, U2, U1)
    trace = bool(os.environ.get("BASS_TRACE"))
    res = run_bass_kernel_spmd(nc, in_maps, list(range(NCORES)), trace=trace)
    LAST_EXEC_NS = res.exec_time_ns
    _CACHE["last_results"] = res

    out = np.zeros((B, C), dtype=np.float64)
    for r in range(NCORES):
        out += res.results[r]["out"].astype(np.float64)
    return out.astype(np.float32)
